# revision 1
# baseline (speedup 1.0000x reference)
"""Trainium2 Bass kernel for a causal single-head attention block.

Reference computation (per batch b):
    q = x @ Wq + bq ; k = x @ Wk + bk ; v = x @ Wv + bv      (x: [S, D])
    logits = q @ k.T  (causal masked), probs = softmax(logits / sqrt(128))
    out = concat([x, probs @ v], axis=-1)                     -> [S, D+128]

Shapes are hardcoded: B=4, S=2048, D=1024, feature size 128, 8 NeuronCores.

Sharding (SPMD, one compiled graph for all 8 cores):
  core c -> batch b = c//2, interleave parity h = c%2.
  Each core computes the 8 query blocks (128 rows each) at global block
  positions {2j + h : j in 0..7} of its batch, and the K/V projection over
  the full 2048-row sequence of that batch.

  To keep the causal block structure identical across cores (SPMD requires
  one instruction stream), the host feeds h=1 cores a pair-swapped column
  order of x^T (global blocks [1,0,3,2,...]).  In local block coordinates
  every core then has: query blocks at even local positions 2j, with valid
  key blocks lk < 2j+2, a triangular causal mask on key slot lk=2j, and a
  slot lk=2j+1 that is fully masked for h=0 / fully valid for h=1.  The two
  128x128 mask tiles are per-core input data.

On-chip scheme (all matmuls bf16, fp32 PSUM accumulation):
  - host passes x^T as bf16 [D, S]  -> projections need no on-chip transpose
  - kT, qT = W.T @ x^T  ([128 feat, rows]) directly in matmul layout
  - logits computed transposed: logitsT[k, q] = kT_blk.T @ qT  -> after the
    masked exp, expT[k, q] is directly the stationary operand of the PV
    matmul (no probs transpose needed)
  - v natural layout via 16 PE transposes of vT, augmented with a ones
    column: read_aug[q, 0:129] = expT.T @ [v | 1] accumulates both the
    attention read and the softmax denominators in one accumulation group
  - normalize with reciprocal * per-partition scale, write fp32
  - the x passthrough half of the output is a DRAM->DRAM DMA of the fp32
    input rows (exact copy)
"""

import math

import numpy as np
import ml_dtypes

import concourse.bass as bass
import concourse.tile as tile
from concourse import bacc, mybir
from concourse.bass_utils import run_bass_kernel_spmd
from concourse.masks import make_identity

N_CORES = 8
B = 4
S = 2048  # sequence length per batch
D = 1024  # model dim
F = 128  # q/k/v feature size
NQT = 8  # local query subtiles of 128 rows
NKT = 16  # key tiles of 128 rows (full sequence)
QROWS = NQT * 128  # 1024 local query rows per core
SCALE = 1.0 / math.sqrt(F)
NEG = -1.0e9

FP32 = mybir.dt.float32
BF16 = mybir.dt.bfloat16
BF16_NP = ml_dtypes.bfloat16

_compiled = {}

# xT DMA chunking (columns of the local sequence): finest first so the
# first projection matmuls start as early as possible.  Host lays xT out
# chunk-major so each chunk is one contiguous DRAM block.
CHUNKS = ((0, 256), (256, 256), (512, 512), (1024, 512), (1536, 512))


def _build():
    nc = bacc.Bacc("TRN2", target_bir_lowering=False, debug=False, num_devices=N_CORES)

    xT_ext = nc.dram_tensor("xT", [D * S], BF16, kind="ExternalInput")
    xq_ext = nc.dram_tensor("xq", [QROWS, D], FP32, kind="ExternalInput")
    wq_ext = nc.dram_tensor("wq", [128, 8, 128], BF16, kind="ExternalInput")
    wk_ext = nc.dram_tensor("wk", [128, 8, 128], BF16, kind="ExternalInput")
    wv_ext = nc.dram_tensor("wv", [128, 8, 128], BF16, kind="ExternalInput")
    bq_ext = nc.dram_tensor("bq", [128, 1], FP32, kind="ExternalInput")
    bk_ext = nc.dram_tensor("bk", [128, 1], FP32, kind="ExternalInput")
    bv_ext = nc.dram_tensor("bv", [128, 1], FP32, kind="ExternalInput")
    mask_ext = nc.dram_tensor("masks", [128, 2, 128], BF16, kind="ExternalInput")
    out_ext = nc.dram_tensor("out", [QROWS, D + F], FP32, kind="ExternalOutput")

    with tile.TileContext(nc) as tc:
        with (
            tc.tile_pool(name="persist", bufs=1) as P,
            tc.tile_pool(name="ps_proj", bufs=2, space="PSUM") as ps_proj,
            tc.tile_pool(name="ps_log", bufs=3, space="PSUM") as ps_log,
            tc.tile_pool(name="ps_tp", bufs=1, space="PSUM") as ps_tp,
            tc.tile_pool(name="ps_read", bufs=2, space="PSUM") as ps_read,
        ):
            # ---- persistent SBUF tiles ----
            xT_sb = P.tile([128, 8, S], BF16)  # [d%128, d//128, s]
            wq_sb = P.tile([128, 8, 128], BF16)
            wk_sb = P.tile([128, 8, 128], BF16)
            wv_sb = P.tile([128, 8, 128], BF16)
            bq_sb = P.tile([128, 1], FP32)
            bk_sb = P.tile([128, 1], FP32)
            bv_sb = P.tile([128, 1], FP32)
            mask_sb = P.tile([128, 2, 128], BF16)
            ident = P.tile([128, 128], BF16)
            zero_sb = P.tile([128, 1], FP32)
            kT_sb = P.tile([128, S], BF16)  # [feat, s]
            qT_sb = P.tile([128, QROWS], BF16)  # [feat, local q]
            vT_sb = P.tile([128, S], BF16)  # [feat, s]
            vaug_sb = P.tile([128, NKT, 132], BF16)  # [s%128, ki, vfeat|1]
            expT_sb = P.tile([128, NKT, QROWS], BF16)  # [s%128, ki, local q]
            read_sb = P.tile([128, NQT, 128], FP32)
            recip_sb = P.tile([128, NQT, 1], FP32)

            # ---- K-proj weights + x^T first (feed the first matmuls).
            # Interleave across the two DMA paths (sync HWDGE / gpsimd
            # SWDGE) so trigger issue and data drains run in parallel;
            # each chunk is a contiguous DRAM block in [p, t, w] order. ----
            xt_dmas = []
            xt_srcs = []
            base = 0
            for off, w in CHUNKS:
                n = 128 * 8 * w
                xt_srcs.append(
                    xT_ext[base:base + n].rearrange("(p t w) -> p t w", p=128, t=8)
                )
                base += n
            for i in (0, 1, 2):
                off, w = CHUNKS[i]
                xt_dmas.append(
                    nc.sync.dma_start(xT_sb[:, :, off:off + w], xt_srcs[i])
                )

            # ---- passthrough out[:, 0:D] = x rows (DRAM -> DRAM, SWDGE).
            # Explicitly held back until the compute-critical xT loads are
            # done so it does not steal SDMA bandwidth from them. ----
            pt_dma = nc.gpsimd.dma_start(out=out_ext[:, 0:D], in_=xq_ext[:])


            # ---- scalar HWDGE ring: K weights + biases first (needed by
            # the first evacuations), then the tail xT chunks in parallel
            # with the sync ring, then the later-needed constants ----
            nc.scalar.dma_start(wk_sb[:], wk_ext[:])
            nc.scalar.dma_start(bk_sb[:], bk_ext[:])
            nc.scalar.dma_start(bv_sb[:], bv_ext[:])
            nc.scalar.dma_start(bq_sb[:], bq_ext[:])
            for i in (3, 4):
                off, w = CHUNKS[i]
                xt_dmas.append(
                    nc.scalar.dma_start(xT_sb[:, :, off:off + w], xt_srcs[i])
                )
            nc.scalar.dma_start(wv_sb[:], wv_ext[:])
            nc.scalar.dma_start(wq_sb[:], wq_ext[:])
            nc.scalar.dma_start(mask_sb[:], mask_ext[:])
            tile.add_dep_helper(
                pt_dma.ins, xt_dmas[2].ins, sync=True, reason="delay passthrough"
            )
            tile.add_dep_helper(
                pt_dma.ins, xt_dmas[4].ins, sync=True, reason="delay passthrough"
            )
            make_identity(nc, ident[:])
            nc.vector.memset(zero_sb[:], 0.0)
            nc.vector.memset(vaug_sb[:, :, 128:129], 1.0)


            # ---- K / V projections over full sequence ----
            # K follows the xT DMA chunking so the first matmul starts as
            # soon as the first 512KB chunk lands; V re-chunks at 512.
            for w_sb, b_sb, dst, chunks in (
                (wk_sb, bk_sb, kT_sb, CHUNKS),
                (wv_sb, bv_sb, vT_sb, [(c * 512, 512) for c in range(4)]),
            ):
                for off, w in chunks:
                    sl = slice(off, off + w)
                    pp = ps_proj.tile([128, w], FP32, tag="proj")
                    for t in range(8):
                        nc.tensor.matmul(
                            pp[:],
                            w_sb[:, t, :],
                            xT_sb[:, t, sl],
                            start=(t == 0),
                            stop=(t == 7),
                        )
                    nc.scalar.activation(
                        dst[:, sl], pp[:], mybir.ActivationFunctionType.Identity,
                        bias=b_sb[:],
                    )

            # ---- v natural layout + ones column ----
            for ki in range(NKT):
                pt = ps_tp.tile([128, 128], BF16, tag="tp")
                nc.tensor.transpose(pt[:], vT_sb[:, ki * 128:(ki + 1) * 128], ident[:])
                nc.vector.tensor_copy(vaug_sb[:, ki, 0:128], pt[:])

            # ---- Q projection (even local blocks only) ----
            for c in range(2):
                pp = ps_proj.tile([128, 512], FP32, tag="proj")
                for t in range(8):
                    qv = xT_sb[:, t, :].rearrange("p (g two f) -> p g two f", two=2, f=128)
                    nc.tensor.matmul(
                        pp[:],
                        wq_sb[:, t, :],
                        qv[:, c * 4:(c + 1) * 4, 0, :],
                        start=(t == 0),
                        stop=(t == 7),
                    )
                nc.scalar.activation(
                    qT_sb[:, c * 512:(c + 1) * 512], pp[:],
                    mybir.ActivationFunctionType.Identity, bias=bq_sb[:],
                )

            # ---- logits^T, mask, exp ----
            for ki in range(NKT):
                qs = 128 * (ki // 2)
                qlen = QROWS - qs
                kb = slice(ki * 128, (ki + 1) * 128)
                off = qs
                first = True
                while off < QROWS:
                    w = min(512, QROWS - off)
                    pl = ps_log.tile([128, w], FP32, tag="log")
                    nc.tensor.matmul(
                        pl[:], kT_sb[:, kb], qT_sb[:, off:off + w],
                        start=True, stop=True,
                    )
                    if first:
                        nc.vector.tensor_add(
                            pl[:, 0:128], pl[:, 0:128], mask_sb[:, ki % 2, :]
                        )
                        first = False
                    nc.scalar.activation(
                        expT_sb[:, ki, off:off + w], pl[:],
                        mybir.ActivationFunctionType.Exp, bias=zero_sb[:],
                        scale=SCALE,
                    )
                    off += w

            # ---- PV + softmax denominators + normalize ----
            for j in range(NQT):
                pr = ps_read.tile([128, 129], FP32, tag="read")
                last = 2 * j + 1
                for ki in range(last + 1):
                    nc.tensor.matmul(
                        pr[:],
                        expT_sb[:, ki, j * 128:(j + 1) * 128],
                        vaug_sb[:, ki, 0:129],
                        start=(ki == 0),
                        stop=(ki == last),
                    )
                nc.vector.reciprocal(recip_sb[:, j, :], pr[:, 128:129])
                nc.vector.tensor_scalar_mul(
                    read_sb[:, j, :], pr[:, 0:128], recip_sb[:, j, :]
                )
                # stream each query subtile's read out as it completes
                out_read = out_ext[:].rearrange("(g p) c -> p g c", p=128)
                nc.gpsimd.dma_start(
                    out=out_read[:, j, D:D + F], in_=read_sb[:, j, :]
                )

    nc.compile()
    return nc


def _get_compiled():
    if "nc" not in _compiled:
        _compiled["nc"] = _build()
    return _compiled["nc"]


def _make_in_maps(inputs, Wq, bq, Wk, bk, Wv, bv):
    x = np.asarray(inputs, dtype=np.float32)
    assert x.shape == (B, S, D)

    def prep_w(w):
        w = np.asarray(w, dtype=np.float32).astype(BF16_NP)
        return np.ascontiguousarray(w.reshape(8, 128, 128).transpose(1, 0, 2))

    wq_np, wk_np, wv_np = prep_w(Wq), prep_w(Wk), prep_w(Wv)
    bq_np = np.asarray(bq, np.float32).reshape(128, 1)
    bk_np = np.asarray(bk, np.float32).reshape(128, 1)
    bv_np = np.asarray(bv, np.float32).reshape(128, 1)

    # masks[k, slot, q]: slot 0 = diagonal block (triangular), slot 1 = the
    # extra block (fully masked for h=0, fully valid for h=1)
    kk = np.arange(128)[:, None]
    qq = np.arange(128)[None, :]
    tri = np.where(qq >= kk, 0.0, NEG).astype(np.float32)
    m_h = []
    for h in range(2):
        other = np.full((128, 128), NEG if h == 0 else 0.0, np.float32)
        m = np.stack([tri, other], axis=1)  # [k, slot, q]
        m_h.append(np.ascontiguousarray(m.astype(BF16_NP)))

    in_maps = []
    for c in range(N_CORES):
        b, h = divmod(c, 2)
        xb = x[b]  # [S, D]
        # local block order: pair-swap for h=1
        order = np.arange(NKT) if h == 0 else (np.arange(NKT) ^ 1)
        xb_local = xb.reshape(NKT, 128, D)[order].reshape(S, D)
        xT_full = xb_local.T.astype(BF16_NP)  # [D, S] = [(t p), s]
        xT_tps = xT_full.reshape(8, 128, S).transpose(1, 0, 2)  # [p, t, s]
        xT = np.concatenate(
            [xT_tps[:, :, off:off + w].reshape(-1) for off, w in CHUNKS]
        )  # chunk-major flat, each chunk contiguous [p, t, w]
        # own query rows = even local blocks = global blocks 2j+h
        qrows = xb_local.reshape(NKT, 128, D)[0::2].reshape(QROWS, D)
        xq = np.ascontiguousarray(qrows, dtype=np.float32)
        in_maps.append(
            {
                "xT": xT,
                "xq": xq,
                "wq": wq_np,
                "wk": wk_np,
                "wv": wv_np,
                "bq": bq_np,
                "bk": bk_np,
                "bv": bv_np,
                "masks": m_h[h],
            }
        )
    return in_maps


def _gather(results):
    out = np.empty((B, S, D + F), dtype=np.float32)
    for c in range(N_CORES):
        b, h = divmod(c, 2)
        oc = results[c]["out"].reshape(NQT, 128, D + F)
        for j in range(NQT):
            g = 2 * j + h
            out[b, g * 128:(g + 1) * 128, :] = oc[j]
    return out


def run(inputs, Wq, bq, Wk, bk, Wv, bv, trace=False):
    """Build (cached), run on 8 cores, gather. Returns (output, results)."""
    nc = _get_compiled()
    in_maps = _make_in_maps(inputs, Wq, bq, Wk, bk, Wv, bv)
    if trace:
        try:
            res = run_bass_kernel_spmd(nc, in_maps, list(range(N_CORES)), trace=True)
            return _gather(res.results), res
        except Exception as e:  # profiling hook unavailable etc.
            print(f"trace run failed ({e!r}); falling back to untraced run")
    res = run_bass_kernel_spmd(nc, in_maps, list(range(N_CORES)))
    return _gather(res.results), res


def kernel(inputs, Wq, bq, Wk, bk, Wv, bv):
    out, _ = run(inputs, Wq, bq, Wk, bk, Wv, bv, trace=False)
    return out



# revision 3
# speedup vs baseline: 18.5875x; 18.5875x over previous
"""Trainium2 Bass kernel for a causal single-head attention block.

Reference computation (per batch b):
    q = x @ Wq + bq ; k = x @ Wk + bk ; v = x @ Wv + bv      (x: [S, D])
    logits = q @ k.T  (causal masked), probs = softmax(logits / sqrt(128))
    out = concat([x, probs @ v], axis=-1)                     -> [S, D+128]

Shapes are hardcoded: B=4, S=2048, D=1024, feature size 128, 8 NeuronCores.

Sharding (SPMD, one compiled graph for all 8 cores):
  core c -> batch b = c//2, interleave parity h = c%2.
  Each core computes the 8 query blocks (128 rows each) at global block
  positions {2j + h : j in 0..7} of its batch, and the K/V projection over
  the full 2048-row sequence of that batch.

  To keep the causal block structure identical across cores (SPMD requires
  one instruction stream), the host feeds h=1 cores a pair-swapped column
  order of x^T (global blocks [1,0,3,2,...]).  In local block coordinates
  every core then has: query blocks at even local positions 2j, with valid
  key blocks lk < 2j+2, a triangular causal mask on key slot lk=2j, and a
  slot lk=2j+1 that is fully masked for h=0 / fully valid for h=1.  The two
  128x128 mask tiles are per-core input data.

On-chip scheme (all matmuls bf16, fp32 PSUM accumulation):
  - host passes x^T as bf16 [D, S]  -> projections need no on-chip transpose
  - kT, qT = W.T @ x^T  ([128 feat, rows]) directly in matmul layout
  - logits computed transposed: logitsT[k, q] = kT_blk.T @ qT  -> after the
    masked exp, expT[k, q] is directly the stationary operand of the PV
    matmul (no probs transpose needed)
  - v natural layout via 16 PE transposes of vT, augmented with a ones
    column: read_aug[q, 0:129] = expT.T @ [v | 1] accumulates both the
    attention read and the softmax denominators in one accumulation group
  - normalize with reciprocal * per-partition scale, write fp32
  - the x passthrough half of the output is a DRAM->DRAM DMA of the fp32
    input rows (exact copy)
"""

import math

import numpy as np
import ml_dtypes

import concourse.bass as bass
import concourse.tile as tile
from concourse import bacc, mybir
from concourse.bass_utils import run_bass_kernel_spmd
from concourse.masks import make_identity

N_CORES = 8
B = 4
S = 2048  # sequence length per batch
D = 1024  # model dim
F = 128  # q/k/v feature size
NQT = 8  # local query subtiles of 128 rows
NKT = 16  # key tiles of 128 rows (full sequence)
QROWS = NQT * 128  # 1024 local query rows per core
SCALE = 1.0 / math.sqrt(F)
NEG = -1.0e9

FP32 = mybir.dt.float32
BF16 = mybir.dt.bfloat16
BF16_NP = ml_dtypes.bfloat16

_compiled = {}

# xT DMA chunking (columns of the local sequence): finest first so the
# first projection matmuls start as early as possible.  Host lays xT out
# chunk-major so each chunk is one contiguous DRAM block.
CHUNKS = ((0, 256), (256, 256), (512, 512), (1024, 512), (1536, 512))


def _build(niter=1):
    nc = bacc.Bacc("TRN2", target_bir_lowering=False, debug=False, num_devices=N_CORES)

    xT_ext = nc.dram_tensor("xT", [D * S], BF16, kind="ExternalInput")
    xq_ext = nc.dram_tensor("xq", [QROWS, D], FP32, kind="ExternalInput")
    wq_ext = nc.dram_tensor("wq", [128, 8, 128], BF16, kind="ExternalInput")
    wk_ext = nc.dram_tensor("wk", [128, 8, 128], BF16, kind="ExternalInput")
    wv_ext = nc.dram_tensor("wv", [128, 8, 128], BF16, kind="ExternalInput")
    bq_ext = nc.dram_tensor("bq", [128, 1], FP32, kind="ExternalInput")
    bk_ext = nc.dram_tensor("bk", [128, 1], FP32, kind="ExternalInput")
    bv_ext = nc.dram_tensor("bv", [128, 1], FP32, kind="ExternalInput")
    mask_ext = nc.dram_tensor("masks", [128, 2, 128], BF16, kind="ExternalInput")
    out_ext = nc.dram_tensor("out", [QROWS, D + F], FP32, kind="ExternalOutput")

    import contextlib

    with tile.TileContext(nc) as tc:
        with (
            tc.tile_pool(name="persist", bufs=1) as P,
            tc.tile_pool(name="ps_proj", bufs=2, space="PSUM") as ps_proj,
            tc.tile_pool(name="ps_log", bufs=3, space="PSUM") as ps_log,
            tc.tile_pool(name="ps_tp", bufs=1, space="PSUM") as ps_tp,
            tc.tile_pool(name="ps_read", bufs=2, space="PSUM") as ps_read,
            tc.For_i(0, niter) if niter > 1 else contextlib.nullcontext(),
        ):
            # ---- persistent SBUF tiles ----
            xT_sb = P.tile([128, 8, S], BF16)  # [d%128, d//128, s]
            wq_sb = P.tile([128, 8, 128], BF16)
            wk_sb = P.tile([128, 8, 128], BF16)
            wv_sb = P.tile([128, 8, 128], BF16)
            bq_sb = P.tile([128, 1], FP32)
            bk_sb = P.tile([128, 1], FP32)
            bv_sb = P.tile([128, 1], FP32)
            mask_sb = P.tile([128, 2, 128], BF16)
            ident = P.tile([128, 128], BF16)
            zero_sb = P.tile([128, 1], FP32)
            kT_sb = P.tile([128, S], BF16)  # [feat, s]
            qT_sb = P.tile([128, QROWS], BF16)  # [feat, local q]
            vT_sb = P.tile([128, S], BF16)  # [feat, s]
            vaug_sb = P.tile([128, NKT, 132], BF16)  # [s%128, ki, vfeat|1]
            expT_sb = P.tile([128, NKT, QROWS], BF16)  # [s%128, ki, local q]
            read_sb = P.tile([128, NQT, 128], FP32)
            recip_sb = P.tile([128, NQT, 1], FP32)

            # ---- K-proj weights + x^T first (feed the first matmuls).
            # Interleave across the two DMA paths (sync HWDGE / gpsimd
            # SWDGE) so trigger issue and data drains run in parallel;
            # each chunk is a contiguous DRAM block in [p, t, w] order. ----
            xt_dmas = []
            xt_srcs = []
            base = 0
            for off, w in CHUNKS:
                n = 128 * 8 * w
                xt_srcs.append(
                    xT_ext[base:base + n].rearrange("(p t w) -> p t w", p=128, t=8)
                )
                base += n
            for i in (0, 1, 2):
                off, w = CHUNKS[i]
                xt_dmas.append(
                    nc.sync.dma_start(xT_sb[:, :, off:off + w], xt_srcs[i])
                )

            # ---- passthrough out[:, 0:D] = x rows (DRAM -> DRAM, SWDGE).
            # Explicitly held back until the compute-critical xT loads are
            # done so it does not steal SDMA bandwidth from them. ----
            pt_dma = nc.gpsimd.dma_start(out=out_ext[:, 0:D], in_=xq_ext[:])


            # ---- scalar HWDGE ring: K weights + biases first (needed by
            # the first evacuations), then the tail xT chunks in parallel
            # with the sync ring, then the later-needed constants ----
            nc.scalar.dma_start(wk_sb[:], wk_ext[:])
            nc.scalar.dma_start(bk_sb[:], bk_ext[:])
            nc.scalar.dma_start(bv_sb[:], bv_ext[:])
            nc.scalar.dma_start(bq_sb[:], bq_ext[:])
            for i in (3, 4):
                off, w = CHUNKS[i]
                xt_dmas.append(
                    nc.scalar.dma_start(xT_sb[:, :, off:off + w], xt_srcs[i])
                )
            nc.scalar.dma_start(wv_sb[:], wv_ext[:])
            nc.scalar.dma_start(wq_sb[:], wq_ext[:])
            nc.scalar.dma_start(mask_sb[:], mask_ext[:])
            tile.add_dep_helper(
                pt_dma.ins, xt_dmas[2].ins, sync=True, reason="delay passthrough"
            )
            tile.add_dep_helper(
                pt_dma.ins, xt_dmas[4].ins, sync=True, reason="delay passthrough"
            )
            make_identity(nc, ident[:])
            nc.vector.memset(zero_sb[:], 0.0)
            nc.vector.memset(vaug_sb[:, :, 128:129], 1.0)


            # ---- K / V projections over full sequence ----
            # K follows the xT DMA chunking so the first matmul starts as
            # soon as the first 512KB chunk lands; V re-chunks at 512.
            for w_sb, b_sb, dst, chunks in (
                (wk_sb, bk_sb, kT_sb, CHUNKS),
                (wv_sb, bv_sb, vT_sb, [(c * 512, 512) for c in range(4)]),
            ):
                for off, w in chunks:
                    sl = slice(off, off + w)
                    pp = ps_proj.tile([128, w], FP32, tag="proj")
                    for t in range(8):
                        nc.tensor.matmul(
                            pp[:],
                            w_sb[:, t, :],
                            xT_sb[:, t, sl],
                            start=(t == 0),
                            stop=(t == 7),
                        )
                    nc.scalar.activation(
                        dst[:, sl], pp[:], mybir.ActivationFunctionType.Identity,
                        bias=b_sb[:],
                    )

            # ---- v natural layout + ones column ----
            for ki in range(NKT):
                pt = ps_tp.tile([128, 128], BF16, tag="tp")
                nc.tensor.transpose(pt[:], vT_sb[:, ki * 128:(ki + 1) * 128], ident[:])
                nc.vector.tensor_copy(vaug_sb[:, ki, 0:128], pt[:])

            # ---- Q projection (even local blocks only) ----
            for c in range(2):
                pp = ps_proj.tile([128, 512], FP32, tag="proj")
                for t in range(8):
                    qv = xT_sb[:, t, :].rearrange("p (g two f) -> p g two f", two=2, f=128)
                    nc.tensor.matmul(
                        pp[:],
                        wq_sb[:, t, :],
                        qv[:, c * 4:(c + 1) * 4, 0, :],
                        start=(t == 0),
                        stop=(t == 7),
                    )
                nc.scalar.activation(
                    qT_sb[:, c * 512:(c + 1) * 512], pp[:],
                    mybir.ActivationFunctionType.Identity, bias=bq_sb[:],
                )

            # ---- logits^T, mask, exp ----
            for ki in range(NKT):
                qs = 128 * (ki // 2)
                qlen = QROWS - qs
                kb = slice(ki * 128, (ki + 1) * 128)
                off = qs
                first = True
                while off < QROWS:
                    w = min(512, QROWS - off)
                    pl = ps_log.tile([128, w], FP32, tag="log")
                    nc.tensor.matmul(
                        pl[:], kT_sb[:, kb], qT_sb[:, off:off + w],
                        start=True, stop=True,
                    )
                    if first:
                        nc.vector.tensor_add(
                            pl[:, 0:128], pl[:, 0:128], mask_sb[:, ki % 2, :]
                        )
                        first = False
                    nc.scalar.activation(
                        expT_sb[:, ki, off:off + w], pl[:],
                        mybir.ActivationFunctionType.Exp, bias=zero_sb[:],
                        scale=SCALE,
                    )
                    off += w

            # ---- PV + softmax denominators + normalize ----
            for j in range(NQT):
                pr = ps_read.tile([128, 129], FP32, tag="read")
                last = 2 * j + 1
                for ki in range(last + 1):
                    nc.tensor.matmul(
                        pr[:],
                        expT_sb[:, ki, j * 128:(j + 1) * 128],
                        vaug_sb[:, ki, 0:129],
                        start=(ki == 0),
                        stop=(ki == last),
                    )
                nc.vector.reciprocal(recip_sb[:, j, :], pr[:, 128:129])
                nc.vector.tensor_scalar_mul(
                    read_sb[:, j, :], pr[:, 0:128], recip_sb[:, j, :]
                )
                # stream each query subtile's read out as it completes
                out_read = out_ext[:].rearrange("(g p) c -> p g c", p=128)
                nc.gpsimd.dma_start(
                    out=out_read[:, j, D:D + F], in_=read_sb[:, j, :]
                )

    nc.compile()
    return nc


def _get_compiled(niter=1):
    key = f"nc{niter}"
    if key not in _compiled:
        _compiled[key] = _build(niter)
    return _compiled[key]


def _make_in_maps(inputs, Wq, bq, Wk, bk, Wv, bv):
    x = np.asarray(inputs, dtype=np.float32)
    assert x.shape == (B, S, D)

    def prep_w(w):
        w = np.asarray(w, dtype=np.float32).astype(BF16_NP)
        return np.ascontiguousarray(w.reshape(8, 128, 128).transpose(1, 0, 2))

    wq_np, wk_np, wv_np = prep_w(Wq), prep_w(Wk), prep_w(Wv)
    bq_np = np.asarray(bq, np.float32).reshape(128, 1)
    bk_np = np.asarray(bk, np.float32).reshape(128, 1)
    bv_np = np.asarray(bv, np.float32).reshape(128, 1)

    # masks[k, slot, q]: slot 0 = diagonal block (triangular), slot 1 = the
    # extra block (fully masked for h=0, fully valid for h=1)
    kk = np.arange(128)[:, None]
    qq = np.arange(128)[None, :]
    tri = np.where(qq >= kk, 0.0, NEG).astype(np.float32)
    m_h = []
    for h in range(2):
        other = np.full((128, 128), NEG if h == 0 else 0.0, np.float32)
        m = np.stack([tri, other], axis=1)  # [k, slot, q]
        m_h.append(np.ascontiguousarray(m.astype(BF16_NP)))

    in_maps = []
    for c in range(N_CORES):
        b, h = divmod(c, 2)
        xb = x[b]  # [S, D]
        # local block order: pair-swap for h=1
        order = np.arange(NKT) if h == 0 else (np.arange(NKT) ^ 1)
        xb_local = xb.reshape(NKT, 128, D)[order].reshape(S, D)
        xT_full = xb_local.T.astype(BF16_NP)  # [D, S] = [(t p), s]
        xT_tps = xT_full.reshape(8, 128, S).transpose(1, 0, 2)  # [p, t, s]
        xT = np.concatenate(
            [xT_tps[:, :, off:off + w].reshape(-1) for off, w in CHUNKS]
        )  # chunk-major flat, each chunk contiguous [p, t, w]
        # own query rows = even local blocks = global blocks 2j+h
        qrows = xb_local.reshape(NKT, 128, D)[0::2].reshape(QROWS, D)
        xq = np.ascontiguousarray(qrows, dtype=np.float32)
        in_maps.append(
            {
                "xT": xT,
                "xq": xq,
                "wq": wq_np,
                "wk": wk_np,
                "wv": wv_np,
                "bq": bq_np,
                "bk": bk_np,
                "bv": bv_np,
                "masks": m_h[h],
            }
        )
    return in_maps


def _gather(results):
    out = np.empty((B, S, D + F), dtype=np.float32)
    for c in range(N_CORES):
        b, h = divmod(c, 2)
        oc = results[c]["out"].reshape(NQT, 128, D + F)
        for j in range(NQT):
            g = 2 * j + h
            out[b, g * 128:(g + 1) * 128, :] = oc[j]
    return out


def run(inputs, Wq, bq, Wk, bk, Wv, bv, trace=False):
    """Build (cached), run on 8 cores, gather. Returns (output, results)."""
    nc = _get_compiled()
    in_maps = _make_in_maps(inputs, Wq, bq, Wk, bk, Wv, bv)
    if trace:
        try:
            res = run_bass_kernel_spmd(nc, in_maps, list(range(N_CORES)), trace=True)
            return _gather(res.results), res
        except Exception as e:  # profiling hook unavailable etc.
            print(f"trace run failed ({e!r}); falling back to untraced run")
    res = run_bass_kernel_spmd(nc, in_maps, list(range(N_CORES)))
    return _gather(res.results), res


def kernel(inputs, Wq, bq, Wk, bk, Wv, bv):
    out, _ = run(inputs, Wq, bq, Wk, bk, Wv, bv, trace=False)
    return out

